# revision 14
# baseline (speedup 1.0000x reference)
"""Attention-pooling kernel for TRN2 (8 NeuronCores, batch-parallel).

Computes, for x:[32,2048,1024], W:[1024,1024], b:[1024], ctx:[1024]:
    h = tanh(x @ W + b); scores = h . ctx
    weights = softmax(scores, axis=seq)
    out = sum_s weights[s] * x[s]          -> [32, 1024]

Sharding: data-parallel over batch, 4 batches per core. The host ships
each core's x shard twice: natural layout [BL*S, E] (pass-2 pooling
moving operand) and pre-transposed [E, BL*S] (pass-1 moving operand) —
transposition is a host-side layout choice of the sharding, which frees
the PE array of the 512 on-device chunk transposes + PSUM evacuations
the previous revision spent ~25% of its cycles on.

Per core: all matmuls run in float32r (full PE rate at N>=512; inputs
DMA'd with a bitcast — the PE truncates f32r operands on read).
Pass 1 computes h^T = W^T x^T per 512-col seq tile directly from the
DMA'd x^T: f32r matmuls accumulating h^T in PSUM, tanh+bias on ScalarE,
and the ctx-dot as an M=1 f32r matmul back on the PE producing scores
[1, S]. Each ctx-dot is emitted one j-group late so the PE's in-order
queue never waits on the tanh. Softmax is unnormalized (exp(s - max);
the 1/Z lands on the pooled vector). Pass 2 pools x-natural with the
transposed weight vector as the stationary operand; it is emitted one
tile into the next batch so the PE never stalls on the softmax chain.
"""

import numpy as np
from contextlib import ExitStack

import concourse.bacc as bacc
import concourse.mybir as mybir
import concourse.tile as tile
from concourse import masks
from concourse.bass_utils import run_bass_kernel_spmd

B, S, E, A = 32, 2048, 1024, 1024
NCORES = 8
BL = B // NCORES          # batches per core
S_TILE = 512
NT = S // S_TILE          # seq tiles per batch
KE = E // 128             # contraction chunks over embed dim
KA = A // 128             # chunks over attention dim
NC2 = S // 128            # S chunks per batch (pass 2)

F32 = mybir.dt.float32
F32R = mybir.dt.float32r
FP16 = mybir.dt.float16
AX = mybir.AxisListType.X
AF = mybir.ActivationFunctionType


def _build():
    nc = bacc.Bacc("TRN2", target_bir_lowering=False, debug=False,
                   num_devices=NCORES)
    x_d = nc.declare_dram_parameter("x", [BL * S, E], FP16, isOutput=False)
    xT_d = nc.declare_dram_parameter("xT", [E, BL * S], FP16, isOutput=False)
    W_d = nc.declare_dram_parameter("W", [E, A], FP16, isOutput=False)
    b_d = nc.declare_dram_parameter("b", [A], F32, isOutput=False)
    c_d = nc.declare_dram_parameter("ctx", [A], F32, isOutput=False)
    o_d = nc.declare_dram_parameter("out", [BL, E], F32, isOutput=True)

    with ExitStack() as ctx:
        tc = ctx.enter_context(tile.TileContext(nc))

        const_pool = ctx.enter_context(tc.tile_pool(name="const", bufs=1))
        xt_pool = ctx.enter_context(tc.tile_pool(name="xt", bufs=3))
        xn_pool = ctx.enter_context(tc.tile_pool(name="xn", bufs=24))
        h_pool = ctx.enter_context(tc.tile_pool(name="h", bufs=3))
        sc_pool = ctx.enter_context(tc.tile_pool(name="scores", bufs=2))
        sm_pool = ctx.enter_context(tc.tile_pool(name="softmax", bufs=1))
        out_pool = ctx.enter_context(tc.tile_pool(name="outs", bufs=1))

        ps_h = ctx.enter_context(tc.tile_pool(name="ps_h", bufs=2, space="PSUM"))
        ps_s = ctx.enter_context(tc.tile_pool(name="ps_s", bufs=2, space="PSUM"))
        ps_t = ctx.enter_context(tc.tile_pool(name="ps_t", bufs=2, space="PSUM"))
        ps_o = ctx.enter_context(tc.tile_pool(name="ps_o", bufs=1, space="PSUM"))

        # ---- constants ----
        ident = const_pool.tile([128, 128], F32)
        masks.make_identity(nc, ident[:])
        ident_h = const_pool.tile([128, 128], FP16)
        nc.vector.tensor_copy(ident_h[:], ident[:])
        neg_ones = const_pool.tile([1, 128], F32)
        nc.gpsimd.memset(neg_ones[:], -1.0)

        W_r = const_pool.tile([128, KE * A], FP16)
        b_sb = const_pool.tile([128, KA], F32)
        ctx_r = const_pool.tile([128, KA], FP16)

        tiles = [(bi, t) for bi in range(BL) for t in range(NT)]

        def dma_xt(bi, t):
            # x^T for seq tile t of batch bi: col block k holds
            # xT[k*128:(k+1)*128, bi*S + t*S_TILE : +S_TILE]
            c0 = bi * S + t * S_TILE
            xt = xt_pool.tile([128, KE * S_TILE], FP16, tag="xt")
            for k in range(KE):
                nc.sync.dma_start(
                    xt[:, k * S_TILE:(k + 1) * S_TILE],
                    xT_d[k * 128:(k + 1) * 128, c0:c0 + S_TILE])
            return xt

        def dma_xn(bi, c):
            r0 = bi * S + c * 128
            xn = xn_pool.tile([128, E], FP16, tag="xn")
            nc.sync.dma_start(xn[:], x_d[r0:r0 + 128, :])
            return xn

        def flush_pass2(pend, filler=False):
            scores_sb, batch_xns, orow = pend

            def keep_warm(n=2):
                # independent matmuls between the serial softmax steps keep
                # the PE duty above the HAM MID threshold so the pool
                # matmuls issue at K=8/8 instead of half clock
                if not filler:
                    return
                for _ in range(n):
                    wp = ps_t.tile([128, 64], F32, tag="tps", name="kw")
                    nc.tensor.matmul(wp[:], ident_h[:], ident_h[:, 0:64],
                                     start=True, stop=True)
            # Transpose the RAW scores (no softmax dependency), then
            # exponentiate in the [128, NC2] layout: the transposes never
            # wait on the softmax chain and the exp is 128-lane-parallel.
            sT = sm_pool.tile([128, NC2], F32, tag="sT")
            for g in range(NC2 // 8):
                tp = ps_t.tile([128, 8], F32, tag="tps")
                for u in range(8):
                    c2 = g * 8 + u
                    nc.tensor.matmul(
                        tp[:, u:u + 1], scores_sb[0:1, c2 * 128:(c2 + 1) * 128],
                        ident[0:1, 0:1],
                        is_transpose=True,
                        start=(u == 0), stop=(u == 7),
                        skip_group_check=True)
                nc.scalar.activation(sT[:, g * 8:(g + 1) * 8], tp[:], AF.Copy)

            # global max of sT; broadcast -max to all partitions with a
            # K=1 matmul against a -1s row
            pm = sm_pool.tile([128, 1], F32, tag="pm")
            nc.vector.reduce_max(pm[:], sT[:], axis=AX)
            keep_warm()
            pmT = ps_t.tile([1, 128], F32, tag="tps")
            nc.tensor.transpose(pmT[:], pm[:], ident[:])
            pmr = sm_pool.tile([1, 128], F32, tag="pmr")
            nc.scalar.activation(pmr[:], pmT[:], AF.Copy)
            keep_warm()
            m_sb = sm_pool.tile([1, 1], F32, tag="m")
            nc.vector.reduce_max(m_sb[:], pmr[:], axis=AX)
            keep_warm()
            mb_ps = ps_t.tile([128, 1], F32, tag="tps")
            nc.tensor.matmul(mb_ps[:], neg_ones[:], m_sb[:],
                             start=True, stop=True)
            mb = sm_pool.tile([128, 1], F32, tag="mb")
            nc.scalar.activation(mb[:], mb_ps[:], AF.Copy)
            keep_warm()

            # exp -> pass-2 stationary operand; accum gives per-partition Z
            pT = sm_pool.tile([128, NC2], FP16, tag="pT")
            zc = sm_pool.tile([128, 1], F32, tag="zc")
            nc.scalar.activation(pT[:], sT[:], AF.Exp, bias=mb[:, 0:1],
                                 accum_out=zc[:])
            keep_warm(4)
            # cross-partition sum of zc -> Z -> 1/Z
            zt = ps_t.tile([1, 128], F32, tag="tps")
            nc.tensor.transpose(zt[:], zc[:], ident[:])
            zrow = sm_pool.tile([1, 128], F32, tag="zrow")
            nc.scalar.activation(zrow[:], zt[:], AF.Copy)
            z_sb = sm_pool.tile([1, 1], F32, tag="z")
            nc.vector.reduce_sum(z_sb[:], zrow[:], axis=AX)
            rz = sm_pool.tile([1, 1], F32, tag="rz")
            nc.vector.reciprocal(rz[:], z_sb[:])

            # pass 2: pooling
            op0 = ps_o.tile([1, 512], F32, tag="op0")
            op1 = ps_o.tile([1, 512], F32, tag="op1")
            for c2 in range(NC2):
                xn = batch_xns[c2]
                nc.tensor.matmul(op0[:], pT[:, c2:c2 + 1], xn[:, 0:512],
                                 start=(c2 == 0), stop=(c2 == NC2 - 1))
                nc.tensor.matmul(op1[:], pT[:, c2:c2 + 1], xn[:, 512:1024],
                                 start=(c2 == 0), stop=(c2 == NC2 - 1))

            ob = out_pool.tile([1, E], F32, tag="ob")
            nc.vector.tensor_scalar_mul(ob[:, 0:512], op0[:], rz[0:1, 0:1])
            nc.vector.tensor_scalar_mul(ob[:, 512:1024], op1[:], rz[0:1, 0:1])
            nc.sync.dma_start(o_d[orow:orow + 1, :], ob[:])

        # prologue ordering: first x^T tile's DMA goes out before the 4MB W
        # load so pass 1 can start ASAP; W chunk k only gates the k-th
        # matmul of the first accumulation group.
        xt_cur = dma_xt(tiles[0][0], tiles[0][1])
        for k in range(KE):
            nc.sync.dma_start(
                W_r[:, k * A:(k + 1) * A],
                W_d[k * 128:(k + 1) * 128, :])
        nc.sync.dma_start(b_sb[:], b_d.rearrange("(j p) -> p j", p=128))
        ctx_f = const_pool.tile([128, KA], F32)
        nc.sync.dma_start(ctx_f[:],
                          c_d.rearrange("(j p) -> p j", p=128))
        nc.vector.tensor_copy(ctx_r[:], ctx_f[:])

        # warm the PE HAM clock-gate with throwaway matmuls while the first
        # DMAs land (the PE would otherwise idle cold and re-throttle)
        warm_scratch = out_pool.tile([128, 128], F32, tag="warm",
                                     name="warm_scratch")
        for w in range(28):
            wp = ps_h.tile([128, 128], F32, tag="hps", name=f"warm{w}")
            nc.tensor.matmul(wp[:], ident_h[:], ident_h[:],
                             start=True, stop=True)
            if w % 14 == 13:
                nc.scalar.activation(warm_scratch[:], wp[:], AF.Copy)

        pending = None          # flush for a finished batch
        pend_ctx = None         # deferred last ctx-dot of the previous tile
        scores_sb = None
        batch_xns = []

        def emit_pend_ctx():
            nonlocal pend_ctx
            if pend_ctx is not None:
                sc_ps_p, j_p, h_p = pend_ctx
                nc.tensor.matmul(sc_ps_p[:], ctx_r[:, j_p:j_p + 1], h_p[:],
                                 start=(j_p == 0), stop=(j_p == KA - 1))
                pend_ctx = None

        prev_copy = None        # (sc_ps, scores_sb, t) awaiting copy-out

        for i, (bi, t) in enumerate(tiles):
            if t == 0:
                scores_sb = sc_pool.tile([1, S], F32, tag="scores")
                batch_xns = []

            if t == 1 and pending is not None:
                flush_pass2(pending)
                pending = None

            nxt = tiles[i + 1] if i + 1 < len(tiles) else None
            if nxt is not None:
                xt_next = dma_xt(nxt[0], nxt[1])
            else:
                xt_next = None
            # pool operands for this batch trickle in during its pass 1
            for c in range(4):
                batch_xns.append(dma_xn(bi, t * 4 + c))

            sc_ps = ps_s.tile([1, S_TILE], F32, tag="scps")
            for j in range(KA):
                hp = ps_h.tile([128, S_TILE], F32, tag="hps")
                for k in range(KE):
                    nc.tensor.matmul(
                        hp[:],
                        W_r[:, k * A + j * 128: k * A + (j + 1) * 128],
                        xt_cur[:, k * S_TILE:(k + 1) * S_TILE],
                        start=(k == 0), stop=(k == KE - 1))
                # previous j's ctx-dot rides behind this group: its tanh
                # has had a full matmul group to complete
                emit_pend_ctx()
                if j == 0 and prev_copy is not None:
                    ps_prev, sb_prev, t_prev = prev_copy
                    nc.vector.tensor_copy(
                        sb_prev[:, t_prev * S_TILE:(t_prev + 1) * S_TILE],
                        ps_prev[:])
                    prev_copy = None
                h_sb = h_pool.tile([128, S_TILE], FP16, tag="h")
                nc.scalar.activation(h_sb[:], hp[:], AF.Tanh,
                                     bias=b_sb[:, j:j + 1])
                pend_ctx = (sc_ps, j, h_sb)

            prev_copy = (sc_ps, scores_sb, t)
            if t == NT - 1:
                pending = (scores_sb, list(batch_xns), bi)

            xt_cur = xt_next

        emit_pend_ctx()
        if prev_copy is not None:
            ps_prev, sb_prev, t_prev = prev_copy
            nc.vector.tensor_copy(
                sb_prev[:, t_prev * S_TILE:(t_prev + 1) * S_TILE], ps_prev[:])
            prev_copy = None
        if pending is not None:
            flush_pass2(pending, filler=True)

    nc.compile()
    return nc


_NC_CACHE = None


def make_in_maps(x, W, b, ctx):
    x = np.ascontiguousarray(np.asarray(x, dtype=np.float32))
    W = np.asarray(W, dtype=np.float32).astype(np.float16)
    b = np.ascontiguousarray(np.asarray(b, dtype=np.float32))
    ctx = np.ascontiguousarray(np.asarray(ctx, dtype=np.float32))
    in_maps = []
    for i in range(NCORES):
        xs = x[i * BL:(i + 1) * BL]                       # [BL, S, E]
        in_maps.append({
            "x": np.ascontiguousarray(
                xs.reshape(BL * S, E).astype(np.float16)),
            "xT": np.ascontiguousarray(
                xs.transpose(2, 0, 1).reshape(E, BL * S)
                .astype(np.float16)),
            "W": W, "b": b, "ctx": ctx,
        })
    return in_maps


def kernel(x, W, b, ctx):
    global _NC_CACHE
    if _NC_CACHE is None:
        _NC_CACHE = _build()
    nc = _NC_CACHE

    in_maps = make_in_maps(x, W, b, ctx)
    res = run_bass_kernel_spmd(nc, in_maps, core_ids=list(range(NCORES)))
    return np.concatenate([res.results[i]["out"] for i in range(NCORES)],
                          axis=0)


if __name__ == "__main__":
    rng = np.random.default_rng(0)
    x = rng.standard_normal((B, S, E), dtype=np.float32)
    W = rng.standard_normal((E, A), dtype=np.float32) / np.sqrt(E)
    b = rng.standard_normal((A,), dtype=np.float32) * 0.01
    c = rng.standard_normal((A,), dtype=np.float32)
    out = kernel(x=x, W=W, b=b, ctx=c)
    print(out.shape, out.dtype)


# revision 18
# speedup vs baseline: 1.0606x; 1.0606x over previous
"""Attention-pooling kernel for TRN2 (8 NeuronCores, batch-parallel).

Computes, for x:[32,2048,1024], W:[1024,1024], b:[1024], ctx:[1024]:
    h = tanh(x @ W + b); scores = h . ctx
    weights = softmax(scores, axis=seq)
    out = sum_s weights[s] * x[s]          -> [32, 1024]

Sharding: data-parallel over batch, 4 batches per core. The host ships
each core's x shard twice: natural layout [BL*S, E] (pass-2 pooling
moving operand) and pre-transposed [E, BL*S] (pass-1 moving operand) —
transposition is a host-side layout choice of the sharding, which frees
the PE array of the 512 on-device chunk transposes + PSUM evacuations
the previous revision spent ~25% of its cycles on.

Per core: all matmuls run in float32r (full PE rate at N>=512; inputs
DMA'd with a bitcast — the PE truncates f32r operands on read).
Pass 1 computes h^T = W^T x^T per 512-col seq tile directly from the
DMA'd x^T: f32r matmuls accumulating h^T in PSUM, tanh+bias on ScalarE,
and the ctx-dot as an M=1 f32r matmul back on the PE producing scores
[1, S]. Each ctx-dot is emitted one j-group late so the PE's in-order
queue never waits on the tanh. Softmax is unnormalized (exp(s - max);
the 1/Z lands on the pooled vector). Pass 2 pools x-natural with the
transposed weight vector as the stationary operand; it is emitted one
tile into the next batch so the PE never stalls on the softmax chain.
"""

import numpy as np
from contextlib import ExitStack

import concourse.bacc as bacc
import concourse.mybir as mybir
import concourse.tile as tile
from concourse import masks
from concourse.bass_utils import run_bass_kernel_spmd

B, S, E, A = 32, 2048, 1024, 1024
NCORES = 8
BL = B // NCORES          # batches per core
S_TILE = 512
NT = S // S_TILE          # seq tiles per batch
KE = E // 128             # contraction chunks over embed dim
KA = A // 128             # chunks over attention dim
NC2 = S // 128            # S chunks per batch (pass 2)

F32 = mybir.dt.float32
F32R = mybir.dt.float32r
FP16 = mybir.dt.float16
AX = mybir.AxisListType.X
AF = mybir.ActivationFunctionType


def _build():
    nc = bacc.Bacc("TRN2", target_bir_lowering=False, debug=False,
                   num_devices=NCORES)
    x_d = nc.declare_dram_parameter("x", [BL * S, E], FP16, isOutput=False)
    xT_d = nc.declare_dram_parameter("xT", [E, BL * S], FP16, isOutput=False)
    W_d = nc.declare_dram_parameter("W", [E, A], FP16, isOutput=False)
    b_d = nc.declare_dram_parameter("b", [A], F32, isOutput=False)
    c_d = nc.declare_dram_parameter("ctx", [A], F32, isOutput=False)
    o_d = nc.declare_dram_parameter("out", [BL, E], F32, isOutput=True)

    with ExitStack() as ctx:
        tc = ctx.enter_context(tile.TileContext(nc))

        const_pool = ctx.enter_context(tc.tile_pool(name="const", bufs=1))
        xt_pool = ctx.enter_context(tc.tile_pool(name="xt", bufs=3))
        xn_pool = ctx.enter_context(tc.tile_pool(name="xn", bufs=24))
        h_pool = ctx.enter_context(tc.tile_pool(name="h", bufs=3))
        sc2_pool = ctx.enter_context(tc.tile_pool(name="sc2", bufs=2))
        sc_pool = ctx.enter_context(tc.tile_pool(name="scores", bufs=2))
        sm_pool = ctx.enter_context(tc.tile_pool(name="softmax", bufs=1))
        out_pool = ctx.enter_context(tc.tile_pool(name="outs", bufs=1))

        ps_h = ctx.enter_context(tc.tile_pool(name="ps_h", bufs=2, space="PSUM"))
        ps_s = ctx.enter_context(tc.tile_pool(name="ps_s", bufs=2, space="PSUM"))
        ps_t = ctx.enter_context(tc.tile_pool(name="ps_t", bufs=2, space="PSUM"))
        ps_o = ctx.enter_context(tc.tile_pool(name="ps_o", bufs=1, space="PSUM"))

        # ---- constants ----
        ident = const_pool.tile([128, 128], F32)
        masks.make_identity(nc, ident[:])
        ident_h = const_pool.tile([128, 128], FP16)
        nc.vector.tensor_copy(ident_h[:], ident[:])
        neg_ones = const_pool.tile([1, 128], F32)
        nc.gpsimd.memset(neg_ones[:], -1.0)

        W_r = const_pool.tile([128, KE * A], FP16)
        b_sb = const_pool.tile([128, KA], F32)
        # diag(ctx) chunks: the ctx-dot runs as full-width fp16 matmuls
        # (diag stationary) instead of M=1 column matmuls — M=1 matmuls at
        # the group boundaries cost ~400ns of issue disruption each
        dg = const_pool.tile([128, KA * 128], FP16)
        ones_f = const_pool.tile([128, 1], F32)
        nc.gpsimd.memset(ones_f[:], 1.0)
        ones_r = const_pool.tile([128, 1], F32R)
        nc.vector.tensor_copy(ones_r[:], ones_f[:])

        tiles = [(bi, t) for bi in range(BL) for t in range(NT)]

        def dma_xt(bi, t):
            # x^T for seq tile t of batch bi: col block k holds
            # xT[k*128:(k+1)*128, bi*S + t*S_TILE : +S_TILE]
            c0 = bi * S + t * S_TILE
            xt = xt_pool.tile([128, KE * S_TILE], FP16, tag="xt")
            for k in range(KE):
                nc.sync.dma_start(
                    xt[:, k * S_TILE:(k + 1) * S_TILE],
                    xT_d[k * 128:(k + 1) * 128, c0:c0 + S_TILE])
            return xt

        def dma_xn(bi, c):
            r0 = bi * S + c * 128
            xn = xn_pool.tile([128, E], FP16, tag="xn")
            nc.sync.dma_start(xn[:], x_d[r0:r0 + 128, :])
            return xn

        def flush_pass2(pend, filler=False):
            scores_sb, batch_xns, orow = pend

            def keep_warm(n=2):
                # independent matmuls between the serial softmax steps keep
                # the PE duty above the HAM MID threshold so the pool
                # matmuls issue at K=8/8 instead of half clock
                if not filler:
                    return
                for _ in range(n):
                    wp = ps_t.tile([128, 64], F32, tag="tps", name="kw")
                    nc.tensor.matmul(wp[:], ident_h[:], ident_h[:, 0:64],
                                     start=True, stop=True)
            # Transpose the RAW scores (no softmax dependency), then
            # exponentiate in the [128, NC2] layout: the transposes never
            # wait on the softmax chain and the exp is 128-lane-parallel.
            sT = sm_pool.tile([128, NC2], F32, tag="sT")
            for g in range(NC2 // 8):
                tp = ps_t.tile([128, 8], F32, tag="tps")
                for u in range(8):
                    c2 = g * 8 + u
                    nc.tensor.matmul(
                        tp[:, u:u + 1], scores_sb[0:1, c2 * 128:(c2 + 1) * 128],
                        ident[0:1, 0:1],
                        is_transpose=True,
                        start=(u == 0), stop=(u == 7),
                        skip_group_check=True)
                nc.scalar.activation(sT[:, g * 8:(g + 1) * 8], tp[:], AF.Copy)

            # global max of sT; broadcast -max to all partitions with a
            # K=1 matmul against a -1s row
            pm = sm_pool.tile([128, 1], F32, tag="pm")
            nc.vector.reduce_max(pm[:], sT[:], axis=AX)
            keep_warm()
            pmT = ps_t.tile([1, 128], F32, tag="tps")
            nc.tensor.transpose(pmT[:], pm[:], ident[:])
            pmr = sm_pool.tile([1, 128], F32, tag="pmr")
            nc.scalar.activation(pmr[:], pmT[:], AF.Copy)
            keep_warm()
            m_sb = sm_pool.tile([1, 1], F32, tag="m")
            nc.vector.reduce_max(m_sb[:], pmr[:], axis=AX)
            keep_warm()
            mb_ps = ps_t.tile([128, 1], F32, tag="tps")
            nc.tensor.matmul(mb_ps[:], neg_ones[:], m_sb[:],
                             start=True, stop=True)
            mb = sm_pool.tile([128, 1], F32, tag="mb")
            nc.scalar.activation(mb[:], mb_ps[:], AF.Copy)
            keep_warm()

            # exp -> pass-2 stationary operand; accum gives per-partition Z
            pT = sm_pool.tile([128, NC2], FP16, tag="pT")
            zc = sm_pool.tile([128, 1], F32, tag="zc")
            nc.scalar.activation(pT[:], sT[:], AF.Exp, bias=mb[:, 0:1],
                                 accum_out=zc[:])
            keep_warm(4)
            # cross-partition sum of zc -> Z -> 1/Z
            zt = ps_t.tile([1, 128], F32, tag="tps")
            nc.tensor.transpose(zt[:], zc[:], ident[:])
            zrow = sm_pool.tile([1, 128], F32, tag="zrow")
            nc.scalar.activation(zrow[:], zt[:], AF.Copy)
            z_sb = sm_pool.tile([1, 1], F32, tag="z")
            nc.vector.reduce_sum(z_sb[:], zrow[:], axis=AX)
            rz = sm_pool.tile([1, 1], F32, tag="rz")
            nc.vector.reciprocal(rz[:], z_sb[:])

            # pass 2: pooling
            op0 = ps_o.tile([1, 512], F32, tag="op0")
            op1 = ps_o.tile([1, 512], F32, tag="op1")
            for c2 in range(NC2):
                xn = batch_xns[c2]
                nc.tensor.matmul(op0[:], pT[:, c2:c2 + 1], xn[:, 0:512],
                                 start=(c2 == 0), stop=(c2 == NC2 - 1))
                nc.tensor.matmul(op1[:], pT[:, c2:c2 + 1], xn[:, 512:1024],
                                 start=(c2 == 0), stop=(c2 == NC2 - 1))

            ob = out_pool.tile([1, E], F32, tag="ob")
            nc.vector.tensor_scalar_mul(ob[:, 0:512], op0[:], rz[0:1, 0:1])
            nc.vector.tensor_scalar_mul(ob[:, 512:1024], op1[:], rz[0:1, 0:1])
            nc.sync.dma_start(o_d[orow:orow + 1, :], ob[:])

        # prologue ordering: first x^T tile's DMA goes out before the 4MB W
        # load so pass 1 can start ASAP; W chunk k only gates the k-th
        # matmul of the first accumulation group.
        xt_cur = dma_xt(tiles[0][0], tiles[0][1])
        for k in range(KE):
            nc.sync.dma_start(
                W_r[:, k * A:(k + 1) * A],
                W_d[k * 128:(k + 1) * 128, :])
        nc.sync.dma_start(b_sb[:], b_d.rearrange("(j p) -> p j", p=128))
        ctx_f = const_pool.tile([128, KA], F32)
        nc.sync.dma_start(ctx_f[:],
                          c_d.rearrange("(j p) -> p j", p=128))
        for j in range(KA):
            nc.scalar.activation(dg[:, j * 128:(j + 1) * 128], ident[:],
                                 AF.Copy, scale=ctx_f[:, j:j + 1])

        # warm the PE HAM clock-gate with throwaway matmuls while the first
        # DMAs land (the PE would otherwise idle cold and re-throttle)
        warm_scratch = out_pool.tile([128, 128], F32, tag="warm",
                                     name="warm_scratch")
        for w in range(28):
            wp = ps_h.tile([128, 128], F32, tag="hps", name=f"warm{w}")
            nc.tensor.matmul(wp[:], ident_h[:], ident_h[:],
                             start=True, stop=True)
            if w % 14 == 13:
                nc.scalar.activation(warm_scratch[:], wp[:], AF.Copy)

        pending = None          # flush for a finished batch
        pend_ctx = None         # deferred last ctx diag-mm of the prev group
        scores_sb = None
        batch_xns = []
        boundary_tasks = []     # deferred per-tile epilogue steps, one per
                                # group boundary so the PE queue never waits

        def emit_pend_ctx():
            nonlocal pend_ctx
            if pend_ctx is not None:
                sc_ps_p, j_p, h_p = pend_ctx
                nc.tensor.matmul(sc_ps_p[:],
                                 dg[:, j_p * 128:(j_p + 1) * 128], h_p[:],
                                 start=(j_p == 0), stop=(j_p == KA - 1))
                pend_ctx = None

        def tile_epilogue(sc_ps_p, sb_prev, t_prev):
            # sc_ps holds per-partition partials ctx_a*h[a,s] summed over
            # a-chunks; evacuate to SBUF, then a ones-column matmul adds
            # the 128 partitions into the scores row.
            sc2 = sc2_pool.tile([128, S_TILE], F32R, tag="sc2")

            def step1():
                nc.scalar.activation(sc2[:], sc_ps_p[:], AF.Copy)

            def step2():
                scrow = ps_t.tile([1, S_TILE], F32, tag="tps")
                nc.tensor.matmul(scrow[:], ones_r[:], sc2[:],
                                 start=True, stop=True)
                nc.vector.tensor_copy(
                    sb_prev[:, t_prev * S_TILE:(t_prev + 1) * S_TILE],
                    scrow[:])

            boundary_tasks.append(step1)
            boundary_tasks.append(step2)

        for i, (bi, t) in enumerate(tiles):
            if t == 0:
                scores_sb = sc_pool.tile([1, S], F32, tag="scores")
                batch_xns = []

            if t == 1 and pending is not None:
                flush_pass2(pending)
                pending = None

            nxt = tiles[i + 1] if i + 1 < len(tiles) else None
            if nxt is not None:
                xt_next = dma_xt(nxt[0], nxt[1])
            else:
                xt_next = None
            # pool operands for this batch trickle in during its pass 1
            for c in range(4):
                batch_xns.append(dma_xn(bi, t * 4 + c))

            sc_ps = ps_s.tile([128, S_TILE], F32, tag="scps")
            for j in range(KA):
                hp = ps_h.tile([128, S_TILE], F32, tag="hps")
                for k in range(KE):
                    nc.tensor.matmul(
                        hp[:],
                        W_r[:, k * A + j * 128: k * A + (j + 1) * 128],
                        xt_cur[:, k * S_TILE:(k + 1) * S_TILE],
                        start=(k == 0), stop=(k == KE - 1))
                # previous j's ctx diag-mm rides behind this group: its
                # tanh has had a full matmul group to complete
                emit_pend_ctx()
                if boundary_tasks:
                    boundary_tasks.pop(0)()
                h_sb = h_pool.tile([128, S_TILE], FP16, tag="h")
                nc.scalar.activation(h_sb[:], hp[:], AF.Tanh,
                                     bias=b_sb[:, j:j + 1])
                pend_ctx = (sc_ps, j, h_sb)

            tile_epilogue(sc_ps, scores_sb, t)
            if t == NT - 1:
                pending = (scores_sb, list(batch_xns), bi)

            xt_cur = xt_next

        emit_pend_ctx()
        while boundary_tasks:
            boundary_tasks.pop(0)()
        if pending is not None:
            flush_pass2(pending, filler=True)

    nc.compile()
    return nc


_NC_CACHE = None


def make_in_maps(x, W, b, ctx):
    x = np.ascontiguousarray(np.asarray(x, dtype=np.float32))
    W = np.asarray(W, dtype=np.float32).astype(np.float16)
    b = np.ascontiguousarray(np.asarray(b, dtype=np.float32))
    ctx = np.ascontiguousarray(np.asarray(ctx, dtype=np.float32))
    in_maps = []
    for i in range(NCORES):
        xs = x[i * BL:(i + 1) * BL]                       # [BL, S, E]
        in_maps.append({
            "x": np.ascontiguousarray(
                xs.reshape(BL * S, E).astype(np.float16)),
            "xT": np.ascontiguousarray(
                xs.transpose(2, 0, 1).reshape(E, BL * S)
                .astype(np.float16)),
            "W": W, "b": b, "ctx": ctx,
        })
    return in_maps


def kernel(x, W, b, ctx):
    global _NC_CACHE
    if _NC_CACHE is None:
        _NC_CACHE = _build()
    nc = _NC_CACHE

    in_maps = make_in_maps(x, W, b, ctx)
    res = run_bass_kernel_spmd(nc, in_maps, core_ids=list(range(NCORES)))
    return np.concatenate([res.results[i]["out"] for i in range(NCORES)],
                          axis=0)


if __name__ == "__main__":
    rng = np.random.default_rng(0)
    x = rng.standard_normal((B, S, E), dtype=np.float32)
    W = rng.standard_normal((E, A), dtype=np.float32) / np.sqrt(E)
    b = rng.standard_normal((A,), dtype=np.float32) * 0.01
    c = rng.standard_normal((A,), dtype=np.float32)
    out = kernel(x=x, W=W, b=b, ctx=c)
    print(out.shape, out.dtype)
